# revision 1
# baseline (speedup 1.0000x reference)
"""Trainium2 Bass kernel for nn_EntInSet (segment_reduce):

    probs = softmax(x, axis=1)                       # [B, C]
    idx   = positions of True in mask, ascending     # [B, C]
    out   = clip(sum_j probs[:, j] * constr[gather idx], 0, 1)   # [B]

Contract: kernel(**inputs) takes FULL inputs, shards rows across 8
NeuronCores (data-parallel on batch), runs one SPMD Bass program, and
returns the FULL [B] output.

Algorithm per 128-row tile group (all engines busy, DMA-bound target):
  1. expv = exp(x) and Z = rowsum(expv) in one ACT pass (accum_out).
     No max-subtraction: x ~ N(0,1) so exp is safely in fp32 range.
  2. cum2[m] = 2 * (#Trues in [0..m]) via DVE tensor_tensor_scan
     (state = (mask + state) + mask). A leading zero column gives the
     exclusive scan 2*pos by reading the buffer shifted one left.
  3. The compaction gathered[j] = constr[row, idx[row, j]] is built by
     GPSIMD local_scatter (per-partition indices) in pos-chunks of 1022
     f32 slots. f32 payloads move as interleaved int16 pairs:
     index stream [2t] = min(2*pos, 2B+2044) - 2B (DVE tensor_scalar,
     saturating chunk-foreign highs into a trash slot), [2t+1] = +1
     (ACT copy). Below-chunk entries go negative (skipped by ucode).
     Within an equal-pos run (Falses then the owning True, ascending m)
     the scatter's last-write-wins leaves the True's value in the slot.
     Per-chunk m-windows are computed host-side from the actual mask.
  4. dot_k = sum_j expv[:, B+j] * gathered_k[:, j] via DVE
     scalar_tensor_tensor accum_out; partials summed, divided by Z
     (DVE reciprocal), clipped to [0, 1].
"""
import numpy as np
import concourse.bacc as bacc
import concourse.mybir as mybir
from concourse.tile import TileContext
from concourse import bass_utils

F32 = mybir.dt.float32
I16 = mybir.dt.int16
U8 = mybir.dt.uint8
ALU = mybir.AluOpType
ACTF = mybir.ActivationFunctionType

N_CORES = 8
CH_SLOTS = 1022           # f32 slots per scatter chunk (dst = 2046 int16 + trash)
NE = 2 * (CH_SLOTS + 1)   # scatter dst size in int16 elems (incl. trash slot)


def _compute_windows(mask_u8: np.ndarray, C: int, pad: int = 16):
    """For each pos-chunk, the global [lo, hi) m-window covering every True
    whose exclusive-pos falls inside the chunk (shared across all rows:
    one SPMD program serves all cores)."""
    _, M = mask_u8.shape
    cum = np.cumsum(mask_u8, axis=1, dtype=np.int32)
    pos_excl = cum - mask_u8
    n_chunk = (C + CH_SLOTS - 1) // CH_SLOTS
    windows = []
    for k in range(n_chunk):
        B = CH_SLOTS * k
        nslot = min(CH_SLOTS, C - B)
        sel = (mask_u8 > 0) & (pos_excl >= B) & (pos_excl < B + nslot)
        any_m = sel.any(axis=0)
        lo = int(np.argmax(any_m))
        hi = int(M - np.argmax(any_m[::-1]))
        lo = max(0, lo - pad) & ~1
        hi = min(M, (hi + pad + 1) & ~1)
        windows.append((B, nslot, lo, hi))
    return windows


def _build(R: int, M: int, C: int, windows):
    nc = bacc.Bacc("TRN2", target_bir_lowering=False, debug=False,
                   num_devices=N_CORES)
    x_d = nc.dram_tensor("x", [R, C], F32, kind="ExternalInput")
    m_d = nc.dram_tensor("mask", [R, M], U8, kind="ExternalInput")
    c_d = nc.dram_tensor("constr", [R, M], F32, kind="ExternalInput")
    o_d = nc.dram_tensor("out", [R, 1], F32, kind="ExternalOutput")

    with TileContext(nc) as tc:
        with tc.tile_pool(name="io", bufs=2) as io_pool, \
             tc.tile_pool(name="work", bufs=2) as work_pool, \
             tc.tile_pool(name="one", bufs=1) as one_pool, \
             tc.tile_pool(name="small", bufs=2) as small_pool:
            for g in range(R // 128):
                rs = slice(g * 128, (g + 1) * 128)
                xt = io_pool.tile([128, C], F32, tag="xt")
                mt = io_pool.tile([128, M], U8, tag="mt")
                ct = io_pool.tile([128, M], F32, tag="ct")
                nc.sync.dma_start(out=xt[:], in_=x_d[rs, :])
                nc.sync.dma_start(out=mt[:], in_=m_d[rs, :])
                nc.sync.dma_start(out=ct[:], in_=c_d[rs, :])

                expv = one_pool.tile([128, C], F32, tag="expv")
                zsum = small_pool.tile([128, 1], F32, tag="zsum")
                nc.scalar.activation(expv[:], xt[:], ACTF.Exp,
                                     accum_out=zsum[:])

                cum2 = work_pool.tile([128, M + 2], I16, tag="cum2")
                nc.vector.memset(cum2[:, 0:1], 0)
                nc.vector.tensor_tensor_scan(
                    out=cum2[:, 1:M + 1], data0=mt[:], data1=mt[:],
                    initial=0.0, op0=ALU.add, op1=ALU.add)

                accs = []
                ct16 = ct[:].bitcast(I16)
                for ki, (B, nslot, lo, hi) in enumerate(windows):
                    W = hi - lo
                    idxb = work_pool.tile([128, 2 * W], I16, tag="idxb")
                    ev = idxb[:].rearrange("p (w two) -> p two w", two=2)
                    nc.vector.tensor_scalar(
                        out=ev[:, 0, :], in0=cum2[:, lo:hi],
                        scalar1=float(2 * B + 2 * CH_SLOTS),
                        scalar2=float(2 * B),
                        op0=ALU.min, op1=ALU.subtract)
                    nc.scalar.activation(ev[:, 1, :], ev[:, 0, :],
                                         ACTF.Copy, bias=1.0)
                    dst = work_pool.tile([128, NE], I16, tag="dst")
                    nc.gpsimd.local_scatter(
                        out_ap=dst[:], data_ap=ct16[:, 2 * lo:2 * hi],
                        idxs_ap=idxb[:], channels=128,
                        num_elems=NE, num_idxs=2 * W)
                    dstf = dst[:].bitcast(F32)
                    prod = one_pool.tile([128, CH_SLOTS], F32, tag="prod")
                    nacc = small_pool.tile([128, 1], F32, tag=f"acc{ki}")
                    nc.vector.scalar_tensor_tensor(
                        out=prod[:, 0:nslot],
                        in0=expv[:, B:B + nslot], scalar=1.0,
                        in1=dstf[:, 0:nslot],
                        op0=ALU.mult, op1=ALU.mult, accum_out=nacc[:])
                    accs.append(nacc)

                acc = accs[0]
                for i, a in enumerate(accs[1:]):
                    s = small_pool.tile([128, 1], F32, tag=f"accs{i}")
                    nc.vector.tensor_tensor(out=s[:], in0=acc[:], in1=a[:],
                                            op=ALU.add)
                    acc = s

                rz = small_pool.tile([128, 1], F32, tag="rz")
                nc.vector.reciprocal(rz[:], zsum[:])
                res = small_pool.tile([128, 1], F32, tag="res")
                nc.vector.tensor_tensor(out=res[:], in0=acc[:], in1=rz[:],
                                        op=ALU.mult)
                resc = small_pool.tile([128, 1], F32, tag="resc")
                nc.vector.tensor_scalar(out=resc[:], in0=res[:],
                                        scalar1=1.0, scalar2=0.0,
                                        op0=ALU.min, op1=ALU.max)
                nc.sync.dma_start(out=o_d[rs, :], in_=resc[:])

    nc.compile()
    return nc


def _run(inputs, trace=False):
    x = np.ascontiguousarray(np.asarray(inputs["x"], dtype=np.float32))
    mask = np.asarray(inputs["mask"])
    constr = np.ascontiguousarray(np.asarray(inputs["constr"],
                                             dtype=np.float32))
    B, C = x.shape
    _, M = mask.shape
    assert B % N_CORES == 0
    R = B // N_CORES
    mask_u8 = np.ascontiguousarray(mask.astype(np.uint8))

    windows = _compute_windows(mask_u8, C)
    nc = _build(R, M, C, windows)

    in_maps = [
        {"x": x[c * R:(c + 1) * R],
         "mask": mask_u8[c * R:(c + 1) * R],
         "constr": constr[c * R:(c + 1) * R]}
        for c in range(N_CORES)
    ]
    res = bass_utils.run_bass_kernel_spmd(
        nc, in_maps, core_ids=list(range(N_CORES)), trace=trace)
    out = np.concatenate([res.results[c]["out"][:, 0]
                          for c in range(N_CORES)])
    return out.astype(np.float32), res


def kernel(**inputs) -> np.ndarray:
    out, _ = _run(inputs, trace=False)
    return out


# revision 9
# speedup vs baseline: 1.6224x; 1.6224x over previous
"""Trainium2 Bass kernel for nn_EntInSet (segment_reduce):

    probs = softmax(x, axis=1)                       # [B, C]
    idx   = positions of True in mask, ascending     # [B, C]
    out   = clip(sum_j probs[:, j] * constr[:, idx[:, j]], 0, 1)   # [B]

Contract: kernel(**inputs) takes FULL inputs, shards rows across 8
NeuronCores (data-parallel on batch), runs one SPMD Bass program, and
returns the FULL [B] output.

Per 128-row tile group:
  1. expv = exp(x), Z = rowsum(expv) in one ACT pass (accum_out).
     No max-subtraction needed: x ~ N(0,1), exp stays in fp32 range.
  2. cfp16 = fp16(constr)  (ACT copy; fp16 keeps 11 mantissa bits, the
     resulting output error ~1e-5 relative, on par with ACT's exp table)
  3. pos = exclusive cumsum of mask along the row (DVE tensor_tensor_scan,
     state = max(mask + state, mask) = mask + state; a leading zero column
     gives the exclusive scan by reading the buffer shifted one left)
  4. The compaction gathered[j] = constr[row, idx[row, j]] runs as a
     GPSIMD local_scatter (per-partition indices) in pos-chunks of 2045
     slots: idx[t] = min(pos, B+2045) - B (DVE tensor_scalar): below-chunk
     entries go negative (skipped by the ucode), above-chunk entries
     saturate into trash slot 2045. Within an equal-pos run (Falses then
     the owning True, ascending m) last-write-wins leaves the True's
     constr value in its slot. Per-chunk m-windows come from the actual
     mask host-side; outside-window correctness is guaranteed by the
     saturation, windows only bound the work.
  5. dot_k: prod = expv[:, B:B+n] * dst[:, 0:n] (DVE TT mult), reduced by
     an in-place ACT Copy with accum_out (DVE's accum path is slow).
  6. out = clip((sum_k dot_k) / Z, 0, 1), DMA'd per group.
"""
import numpy as np
import concourse.bacc as bacc
import concourse.mybir as mybir
from concourse.tile import TileContext
from concourse import bass_utils

F32 = mybir.dt.float32
F16 = mybir.dt.float16
I16 = mybir.dt.int16
U8 = mybir.dt.uint8
ALU = mybir.AluOpType
ACTF = mybir.ActivationFunctionType

N_CORES = 8
CH_SLOTS = 2045           # valid slots per scatter chunk
NE = CH_SLOTS + 1         # dst elems: valid slots + 1 trash slot (even)


def _compute_windows(mask_u8: np.ndarray, C: int, pad: int = 16):
    """For each pos-chunk, the global [lo, hi) m-window covering every True
    whose exclusive-pos falls inside the chunk (shared across all rows:
    one SPMD program serves all cores)."""
    _, M = mask_u8.shape
    cum = np.cumsum(mask_u8, axis=1, dtype=np.int32)
    pos_excl = cum - mask_u8
    n_chunk = (C + CH_SLOTS - 1) // CH_SLOTS
    windows = []
    for k in range(n_chunk):
        B = CH_SLOTS * k
        nslot = min(CH_SLOTS, C - B)
        sel = (mask_u8 > 0) & (pos_excl >= B) & (pos_excl < B + nslot)
        any_m = sel.any(axis=0)
        lo = int(np.argmax(any_m))
        hi = int(M - np.argmax(any_m[::-1]))
        lo = max(0, lo - pad) & ~1
        hi = min(M, (hi + pad + 1) & ~1)
        windows.append((B, nslot, lo, hi))
    return windows


def _build(R: int, M: int, C: int, windows):
    nc = bacc.Bacc("TRN2", target_bir_lowering=False, debug=False,
                   num_devices=N_CORES)
    x_d = nc.dram_tensor("x", [R, C], F32, kind="ExternalInput")
    m_d = nc.dram_tensor("mask", [R, M], U8, kind="ExternalInput")
    c_d = nc.dram_tensor("constr", [R, M], F32, kind="ExternalInput")
    o_d = nc.dram_tensor("out", [R, 1], F32, kind="ExternalOutput")
    n_chunk = len(windows)

    with TileContext(nc) as tc:
        with tc.tile_pool(name="io", bufs=2) as io_pool, \
             tc.tile_pool(name="work", bufs=2) as work_pool, \
             tc.tile_pool(name="one", bufs=1) as one_pool, \
             tc.tile_pool(name="small", bufs=2) as small_pool:
            for g in range(R // 128):
                rs = slice(g * 128, (g + 1) * 128)
                xt = io_pool.tile([128, C], F32, tag="xt")
                mt = io_pool.tile([128, M], U8, tag="mt")
                # casting DMA (SWDGE): f32 DRAM -> fp16 SBUF
                c16 = work_pool.tile([128, M], F16, tag="c16")
                nc.sync.dma_start(out=xt[:], in_=x_d[rs, :])
                nc.sync.dma_start(out=mt[:], in_=m_d[rs, :])
                nc.gpsimd.dma_start(out=c16[:], in_=c_d[rs, :])

                expv = work_pool.tile([128, C], F32, tag="expv")
                zsum = small_pool.tile([128, 1], F32, tag="zsum")
                nc.scalar.activation(expv[:], xt[:], ACTF.Exp,
                                     accum_out=zsum[:])

                cum = work_pool.tile([128, M + 2], I16, tag="cum")
                nc.vector.memset(cum[:, 0:1], 0)
                nc.vector.tensor_tensor_scan(
                    out=cum[:, 1:M + 1], data0=mt[:], data1=mt[:],
                    initial=0.0, op0=ALU.add, op1=ALU.max)

                dots = small_pool.tile([128, n_chunk], F32, tag="dots")
                for ki, (B, nslot, lo, hi) in enumerate(windows):
                    W = hi - lo
                    lam = work_pool.tile([128, W], I16, tag="lam")
                    nc.vector.tensor_scalar(
                        out=lam[:], in0=cum[:, lo:hi],
                        scalar1=float(B + CH_SLOTS),
                        scalar2=float(B),
                        op0=ALU.min, op1=ALU.subtract)
                    dst = work_pool.tile([128, NE], F16, tag="dst")
                    nc.gpsimd.local_scatter(
                        out_ap=dst[:], data_ap=c16[:, lo:hi],
                        idxs_ap=lam[:], channels=128,
                        num_elems=NE, num_idxs=W)
                    prod = one_pool.tile([128, CH_SLOTS], F32, tag="prod")
                    nc.vector.tensor_tensor(
                        out=prod[:, 0:nslot],
                        in0=expv[:, B:B + nslot], in1=dst[:, 0:nslot],
                        op=ALU.mult)
                    nc.scalar.activation(prod[:, 0:nslot], prod[:, 0:nslot],
                                         ACTF.Copy,
                                         accum_out=dots[:, ki:ki + 1])

                acc = small_pool.tile([128, 1], F32, tag="acc")
                nc.vector.tensor_reduce(
                    out=acc[:], in_=dots[:], axis=mybir.AxisListType.X,
                    op=ALU.add)
                rz = small_pool.tile([128, 1], F32, tag="rz")
                nc.vector.reciprocal(rz[:], zsum[:])
                res = small_pool.tile([128, 1], F32, tag="res")
                nc.vector.tensor_tensor(out=res[:], in0=acc[:], in1=rz[:],
                                        op=ALU.mult)
                resc = small_pool.tile([128, 1], F32, tag="resc")
                nc.vector.tensor_scalar(out=resc[:], in0=res[:],
                                        scalar1=1.0, scalar2=0.0,
                                        op0=ALU.min, op1=ALU.max)
                nc.sync.dma_start(out=o_d[rs, :], in_=resc[:])

    nc.compile()
    return nc


def _run(inputs, trace=False):
    x = np.ascontiguousarray(np.asarray(inputs["x"], dtype=np.float32))
    mask = np.asarray(inputs["mask"])
    constr = np.ascontiguousarray(np.asarray(inputs["constr"],
                                             dtype=np.float32))
    B, C = x.shape
    _, M = mask.shape
    assert B % N_CORES == 0
    R = B // N_CORES
    mask_u8 = np.ascontiguousarray(mask.astype(np.uint8))

    windows = _compute_windows(mask_u8, C)
    nc = _build(R, M, C, windows)

    in_maps = [
        {"x": x[c * R:(c + 1) * R],
         "mask": mask_u8[c * R:(c + 1) * R],
         "constr": constr[c * R:(c + 1) * R]}
        for c in range(N_CORES)
    ]
    res = bass_utils.run_bass_kernel_spmd(
        nc, in_maps, core_ids=list(range(N_CORES)), trace=trace)
    out = np.concatenate([res.results[c]["out"][:, 0]
                          for c in range(N_CORES)])
    return out.astype(np.float32), res


def kernel(**inputs) -> np.ndarray:
    out, _ = _run(inputs, trace=False)
    return out
